# revision 26
# baseline (speedup 1.0000x reference)
"""BiAttention (BiDAF) Trainium2 Bass kernel — 8 NeuronCores, sequence-
parallel over the context axis.

kernel(context [16384,100] f32, question [4096,100] f32, kernel [300] f32)
  -> G [16384, 400] f32  (concat: ctx | U_A | ctx*U_A | ctx*H_A)

Single-pass design: S is computed ONCE per core (2048 ctx rows) in fp16 on
the PE (1 cyc/col), with a host-precomputed per-row stability offset
C_i = c.w1 + 4*||w2 + c*w3|| folded in as an extra contraction row, so ACT
exps PSUM directly (batched FD=1536 across 3 banks). The exp'd P tiles
(bf16) feed the U_A^T accumulation on the PE and a DVE running-max; the
Q2C row max is max_j P = exp(m_i - C_i), reduced across q-partitions on
GPSIMD, rescaled by E_i = exp(C_i - G0) (host input, exact cancellation of
the fp16 rounding of C), and combined across cores with one 102-float
AllGather.
"""
import sys

sys.path.insert(0, "/opt/trn_rl_repo")
from contextlib import ExitStack

import numpy as np

import concourse.bass as bass
import concourse.tile as tile
from concourse import bass_isa, mybir


def split_multi_waits(nc):
    """This walrus build rejects instructions with >1 sync wait. Hoist extra
    waits onto single-wait EventSemaphore nops on the same engine (engines
    execute in order, so N sequential single waits == one N-way wait)."""
    n_split = 0
    counter = [0]

    def make_nop(engine, wait):
        counter[0] += 1
        inst = mybir.InstEventSemaphore(
            name=f"I-waitsplit-{counter[0]}", ins=[], outs=[])
        inst.engine = engine
        inst.sync_info = mybir.SyncInfo(on_wait=[wait], on_update=[])
        return inst

    for f in nc.m.functions:
        for blk in f.blocks:
            changed = False
            new_insts = []
            for inst in blk.instructions:
                si = inst.sync_info
                if si is not None and si.on_wait and len(si.on_wait) > 1:
                    waits = list(si.on_wait)
                    for w in waits[:-1]:
                        new_insts.append(make_nop(inst.engine, w))
                    si.on_wait = [waits[-1]]
                    n_split += 1
                    changed = True
                new_insts.append(inst)
            if changed:
                blk.instructions[:] = new_insts
    return n_split


F32 = mybir.dt.float32
F16 = mybir.dt.float16
BF16 = mybir.dt.bfloat16
EXP = mybir.ActivationFunctionType.Exp
COPY = mybir.ActivationFunctionType.Copy

N_CORES = 8
D = 100
R = 2048          # ctx rows per core
M = 4096          # question rows
P = 128           # partitions
NCH = R // P      # 16 ctx chunks
QC = M // P       # 32 q chunks
NT = R // 512     # 4 ctx tiles
CPT = 4           # ctx chunks per tile
# q-chunks per ACT batch (3 banks double-buffered + UA + tiny = 8 banks)
BATCHES = [3] * 10 + [2]


def build_bass():
    nc = bass.Bass("TRN2", target_bir_lowering=False, debug=False,
                   num_devices=N_CORES)
    ctx_in = nc.dram_tensor("ctx", [R, D], F32, kind="ExternalInput").ap()
    caugT_in = nc.dram_tensor("caugT", [102, R], F16, kind="ExternalInput").ap()
    qaugT_in = nc.dram_tensor("qaugT", [102, M], F16, kind="ExternalInput").ap()
    qnr_in = nc.dram_tensor("qnr", [M, D], BF16, kind="ExternalInput").ap()
    efac_in = nc.dram_tensor("efac", [P, NCH], F32, kind="ExternalInput").ap()
    id_in = nc.dram_tensor("ident", [P, P], BF16, kind="ExternalInput").ap()
    g_out = nc.dram_tensor("g", [R, 4 * D], F32, kind="ExternalOutput").ap()

    with tile.TileContext(nc) as tc:
        with ExitStack() as ex:
            build_body(nc, tc, ex, ctx_in, caugT_in, qaugT_in, qnr_in,
                       efac_in, id_in, g_out)
    return nc


def build_body(nc, tc, ex, ctx_in, caugT_in, qaugT_in, qnr_in, efac_in,
               id_in, g_out):
    sing = ex.enter_context(tc.tile_pool(name="sing", bufs=1))
    # 22 bufs: tiles 0-1 rotate through, tiles 2-3's P tiles persist so
    # their U_A matmuls can run after the AllGather launches (hiding the
    # ~40us collective latency behind real work).
    ptt_pool = ex.enter_context(tc.tile_pool(name="ptt", bufs=22))
    uat_pool = ex.enter_context(tc.tile_pool(name="uat", bufs=2))
    g12_pool = ex.enter_context(tc.tile_pool(name="g12", bufs=4))
    # PSUM: S batches 2x[128,1536](3 banks each) + UA [101,512](1) + tiny(1)
    sp = ex.enter_context(tc.tile_pool(name="sp", bufs=2, space="PSUM"))
    uap = ex.enter_context(tc.tile_pool(name="uap", bufs=1, space="PSUM"))
    tp = ex.enter_context(tc.tile_pool(name="tp", bufs=1, space="PSUM"))
    dram = ex.enter_context(tc.tile_pool(name="dram", bufs=1, space="DRAM"))

    # ---- persistent SBUF ----
    qaugT = sing.tile([102, M], F16)       # 0..99 (w3*q)^T | 100 q2 | 101 ones
    caugT = sing.tile([102, R], F16)       # 0..99 ctx^T | 100 ones | 101 b16
    qaugN = sing.tile([P, QC, 101], BF16)  # q natural chunks + ones col
    ctxn = sing.tile([P, NCH, 100], F32)   # ctx natural chunks (G muls)
    ctxnr = sing.tile([P, NCH, 101], BF16)  # ctx natural + ones col (hp lhsT)
    efac = sing.tile([P, NCH], F32)        # E_i = exp(Ceff - G0)
    tid = sing.tile([P, P], BF16)
    tpn = sing.tile([P, NCH], F32)         # t' natural (exp'd row maxes)
    bnf = sing.tile([P, NCH], F32)
    bnum = sing.tile([P, NCH], BF16)       # t' * E, bf16 for hp MMs
    uan = sing.tile([P, NCH, 101], BF16)   # U_A^T unnorm natural + Z col
    rzs = sing.tile([P, NCH], F32)         # 1/Z per chunk
    rm3 = sing.tile([P, 1536], BF16)       # running max, 3 chunk lanes
    rmx = sing.tile([P, 512], BF16)        # folded running max per tile
    hprow = sing.tile([1, 101], F32)       # accumulated b partials (row)
    ones1 = sing.tile([1, P], F32)
    ones8 = sing.tile([N_CORES, 1], F32)
    hB = sing.tile([P, D], F32)
    g3big = sing.tile([P, NCH, D], F32)
    dummy = sing.tile([1, 1], F32)

    cc_in = dram.tile([1, 102], F32)
    cc_out = dram.tile([N_CORES, 102], F32)

    # ---- input loads; critical path (caugT tile0 + qaugT chunks) on the
    # sync queue, the rest on the scalar/gpsimd queues in parallel ----
    nc.vector.memset(qaugN[:, :, 100:101], 1.0)
    nc.sync.dma_start(out=caugT[:, 0:512], in_=caugT_in[:, 0:512])
    nc.sync.dma_start(out=qaugT[:, 0:1024], in_=qaugT_in[:, 0:1024])
    qnr_r = qnr_in.rearrange("(c p) d -> p c d", p=P)
    nc.sync.dma_start(out=qaugN[:, 0:8, 0:100], in_=qnr_r[:, 0:8, :])
    for qq in range(1, 4):
        nc.sync.dma_start(out=qaugT[:, qq * 1024:(qq + 1) * 1024],
                          in_=qaugT_in[:, qq * 1024:(qq + 1) * 1024])
        nc.sync.dma_start(out=qaugN[:, qq * 8:(qq + 1) * 8, 0:100],
                          in_=qnr_r[:, qq * 8:(qq + 1) * 8, :])
    for t in range(1, NT):
        nc.sync.dma_start(out=caugT[:, t * 512:(t + 1) * 512],
                          in_=caugT_in[:, t * 512:(t + 1) * 512])
    # non-critical loads AFTER the matmul operands, on the same queue, so
    # they don't steal HBM bandwidth from the stream-critical DMAs
    nc.scalar.dma_start(out=tid[:], in_=id_in[:])
    nc.sync.dma_start(
        out=ctxn[:],
        in_=ctx_in.rearrange("(c p) d -> p c d", p=P))
    nc.sync.dma_start(out=efac[:], in_=efac_in[:])
    nc.vector.memset(ctxnr[:, :, 100:101], 1.0)
    nc.vector.tensor_copy(ctxnr[:, :, 0:100], ctxn[:])
    nc.vector.memset(ones1[:], 1.0)
    nc.vector.memset(ones8[:], 1.0)
    nc.vector.memset(hprow[:], 0.0)
    nc.vector.memset(dummy[:], 0.0)
    # preload the exp table set early (hidden behind input DMAs)
    nc.scalar.activation(dummy[:], dummy[:], EXP)

    # G cols 0:100 = context verbatim (DRAM->DRAM, after the input loads)
    nc.sync.dma_start(out=g_out[:, 0:D], in_=ctx_in[:])

    def do_batch(t, b, qc0, nb, with_ua):
        """S matmuls + exp (+ UA matmuls if inline) + running max for one
        ACT batch of nb q-chunks on ctx tile t."""
        w = nb * 512
        sb = sp.tile([P, 1536], F32, tag="sb", name=f"sb_{t}_{b}")
        for j in range(nb):
            nc.tensor.matmul(sb[:, j * 512:(j + 1) * 512],
                             qaugT[:, (qc0 + j) * P:(qc0 + j + 1) * P],
                             caugT[:, t * 512:(t + 1) * 512],
                             start=True, stop=True)
        ptt = ptt_pool.tile([P, 1536], BF16, tag="ptt", name=f"ptt_{t}_{b}")
        nc.scalar.activation(ptt[:, 0:w], sb[:, 0:w], EXP)
        if with_ua:
            do_ua(t, b, qc0, nb, ptt)
        if b == 0:
            nc.vector.tensor_copy(rm3[:], ptt[:])
        elif nb == 3:
            nc.vector.tensor_max(rm3[:], rm3[:], ptt[:])
        return ptt

    def do_ua(t, b, qc0, nb, ptt):
        for j in range(nb):
            nc.tensor.matmul(uaps[0], qaugN[:, qc0 + j, :],
                             ptt[:, j * 512:(j + 1) * 512],
                             start=(qc0 + j == 0), stop=(qc0 + j == QC - 1))

    def tile_tail_a(t, ptt_last, with_ua):
        """Immediate tail: fold running max (+ cast UA if inline)."""
        nc.vector.tensor_max(rmx[:], rm3[:, 0:512], rm3[:, 512:1024])
        nc.vector.tensor_max(rmx[:], rmx[:], rm3[:, 1024:1536])
        nc.vector.tensor_max(rmx[:], rmx[:], ptt_last[:, 0:512])
        nc.vector.tensor_max(rmx[:], rmx[:], ptt_last[:, 512:1024])
        rmc = sing.tile([P, 512], BF16, tag=f"rmc{t}", name=f"rmc_{t}")
        nc.vector.tensor_copy(rmc[:], rmx[:])
        uat = None
        if with_ua:
            uat = uat_pool.tile([101, 512], BF16, tag="uat", name=f"uat_{t}")
            nc.vector.tensor_copy(uat[:], uaps[0])
        return rmc, uat

    def tchain_units(t, rmc):
        """t' natural recovery (transpose + q-lane max) + b partials, as
        schedulable units."""
        def rmt_unit(ci):
            def f():
                rmt = tp.tile([P, P], BF16, tag="tiny",
                              name=f"rmt_{t}_{ci}")
                nc.tensor.transpose(rmt[:], rmc[:, ci * P:(ci + 1) * P],
                                    tid[:])
                nc.vector.reduce_max(tpn[:, t * CPT + ci:t * CPT + ci + 1],
                                     rmt[:], axis=mybir.AxisListType.X)
            return f

        def hp_unit():
            sl = slice(t * CPT, t * CPT + CPT)
            nc.vector.tensor_mul(bnf[:, sl], tpn[:, sl], efac[:, sl])
            nc.vector.tensor_copy(bnum[:, sl], bnf[:, sl])
            # row-form partial (lhsT=bnum) so the collective staging DMA
            # is one contiguous 404B transfer from a single partition
            hpp = tp.tile([1, 101], F32, tag="tiny", name=f"hpp_{t}")
            for ci in range(CPT):
                cc = t * CPT + ci
                nc.tensor.matmul(hpp[:], bnum[:, cc:cc + 1], ctxnr[:, cc, :],
                                 start=(ci == 0), stop=(ci == CPT - 1))
            nc.vector.tensor_add(hprow[:], hprow[:], hpp[:])

        return [rmt_unit(ci) for ci in range(CPT)] + [hp_unit]

    def ua_unit(t, ci, uat, dmaq):
        cc = t * CPT + ci
        uanps = tp.tile([P, 101], BF16, tag="tiny", name=f"uanps_{t}_{ci}")
        nc.tensor.transpose(uanps[:], uat[:, ci * P:(ci + 1) * P],
                            tid[0:101, 0:101])
        nc.vector.tensor_copy(uan[:, cc, :], uanps[:])
        nc.vector.reciprocal(rzs[:, cc:cc + 1], uan[:, cc, 100:101])
        g12 = g12_pool.tile([P, 2 * D], F32, tag="g12", name=f"g12_{t}_{ci}")
        nc.vector.tensor_scalar_mul(g12[:, 0:D], uan[:, cc, 0:D],
                                    rzs[:, cc:cc + 1])
        nc.vector.tensor_mul(g12[:, D:2 * D], ctxn[:, cc, :], g12[:, 0:D])
        return dmaq.dma_start(out=g_out[cc * P:(cc + 1) * P, D:3 * D],
                              in_=g12[:])

    def uachain_units(t, uat, dmaq=None):
        q = dmaq or nc.sync
        return [
            (lambda ci=ci: ua_unit(t, ci, uat, q)) for ci in range(CPT)]

    # ---- main pipeline: S/exp stream for all tiles; UA inline only for
    # tiles 0-1. Tiles 2-3 keep their P tiles and run UA after the
    # AllGather launches (pinned there so the scheduler can't hoist them).
    # Deferred per-tile tail work is spread one unit per batch so the
    # single tiny PSUM bank never stalls the in-order engine queues.
    from collections import deque
    uaps = [None]
    units = deque()
    ptts = {}         # (t, b) -> (qc0, nb, ptt) for deferred-UA tiles
    for t in range(NT):
        with_ua = t < 2
        if with_ua:
            uaps[0] = uap.tile([101, 512], F32, tag="uaps", name=f"uaps_{t}")
        qc0 = 0
        for b, nb in enumerate(BATCHES):
            ptt = do_batch(t, b, qc0, nb, with_ua)
            if not with_ua:
                ptts[(t, b)] = (qc0, nb, ptt)
            qc0 += nb
            if b >= 2 and units:
                units.popleft()()
        rmc, uat = tile_tail_a(t, ptt, with_ua)
        if t < NT - 1:
            units.extend(tchain_units(t, rmc))
        else:
            for f in tchain_units(t, rmc):
                f()
        if with_ua:
            units.extend(uachain_units(t, uat))
    while units:
        units.popleft()()

    # ---- launch the AllGather as soon as all b partials exist ----
    nc.gpsimd.dma_start(out=cc_in[0:1, 0:101], in_=hprow[:])
    cc_inst = nc.gpsimd.collective_compute(
        "AllGather", mybir.AluOpType.bypass,
        replica_groups=[list(range(N_CORES))],
        ins=[cc_in.opt()], outs=[cc_out.opt()])

    # ---- post-launch: UA for tile 3 (overlapping the collective);
    # ordering-only edge keeps the scheduler from hoisting it ahead of
    # the collective trigger.
    from concourse.tile_rust import add_dep_helper as _adh
    last3 = None
    first_ua = None
    uat3_cast = None
    for t in (2, 3):
        uaps[0] = uap.tile([101, 512], F32, tag="uaps", name=f"uaps_{t}")
        for b, nb in enumerate(BATCHES):
            qc0, nb_, ptt = ptts[(t, b)]
            for j in range(nb_):
                mm = nc.tensor.matmul(
                    uaps[0], qaugN[:, qc0 + j, :],
                    ptt[:, j * 512:(j + 1) * 512],
                    start=(qc0 + j == 0), stop=(qc0 + j == QC - 1))
                if first_ua is None:
                    first_ua = mm
        uat = uat_pool.tile([101, 512], BF16, tag="uat", name=f"uat_{t}")
        uat3_cast = nc.vector.tensor_copy(uat[:], uaps[0])
        for f in uachain_units(t, uat, nc.scalar if t == 3 else nc.sync):
            last3 = f()
    _adh(first_ua.ins, cc_inst.ins, sync=False,
         reason="deferred UA stays after collective trigger")

    # ---- combine after AllGather ----
    # Pin the combine's first load behind tile 3's last G12 store so the
    # scheduler cannot stall engines on the collective mid-stream.
    from concourse.tile_rust import add_dep_helper as _adh
    agm = sing.tile([N_CORES, 102], F32)
    d1 = nc.sync.dma_start(out=agm[:], in_=cc_out[:])
    _adh(d1.ins, uat3_cast.ins, sync=True, reason="combine after tile3 UA")
    hsps = tp.tile([1, 102], F32, tag="tiny", name="hsps")
    nc.tensor.matmul(hsps[:], ones8[:], agm[:], start=True, stop=True)
    hsum = sing.tile([1, 102], F32)
    nc.scalar.activation(hsum[:], hsps[:], COPY)
    rzh = sing.tile([1, 1], F32)
    nc.vector.reciprocal(rzh[:], hsum[:, 100:101])
    hrow = sing.tile([1, D], F32)
    nc.vector.tensor_scalar_mul(hrow[:], hsum[:, 0:D], rzh[:])
    hbps = tp.tile([P, D], F32, tag="tiny", name="hbps")
    nc.tensor.matmul(hbps[:], ones1[:], hrow[:], start=True, stop=True)
    nc.scalar.activation(hB[:], hbps[:], COPY)
    # per-chunk mul (split across DVE and GpSimd) + store on alternating
    # DMA queues so the strided 400B-line writes overlap the muls
    for cc in range(NCH):
        eng = nc.vector if cc % 2 == 0 else nc.gpsimd
        eng.tensor_mul(g3big[:, cc, :], ctxn[:, cc, :], hB[:])
        q = nc.sync if cc % 2 == 0 else nc.scalar
        q.dma_start(
            out=g_out[cc * P:(cc + 1) * P, 3 * D:4 * D],
            in_=g3big[:, cc, :])
    return cc_inst


_nc_cache = None


def _get_nc():
    global _nc_cache
    if _nc_cache is None:
        _nc_cache = build_bass()
        split_multi_waits(_nc_cache)
    return _nc_cache


def _prep_inputs(inputs):
    import ml_dtypes

    context = np.ascontiguousarray(inputs["context"], dtype=np.float32)
    question = np.ascontiguousarray(inputs["question"], dtype=np.float32)
    kern = np.ascontiguousarray(inputs["kernel"], dtype=np.float32)
    w1, w2, w3 = kern[:D], kern[D:2 * D], kern[2 * D:]

    # shared across cores
    qaugT = np.empty((102, M), np.float16)
    qaugT[0:D] = (question * w3[None, :]).T
    qaugT[D] = question @ w2
    qaugT[D + 1] = 1.0
    qaugT = np.ascontiguousarray(qaugT)
    qnr = np.ascontiguousarray(question.astype(ml_dtypes.bfloat16))
    ident = np.eye(P, dtype=ml_dtypes.bfloat16)

    # per-row stability offset C_i = c.w1 + 4*||w2 + c*w3||, fp16-rounded
    # via the bias row; E uses the rounded value so the rounding cancels.
    cw1 = context @ w1
    v = w2[None, :] + context * w3[None, :]
    vn = np.sqrt((v * v).sum(axis=1))
    C = cw1 + np.float32(4.0) * vn
    bias16 = (cw1 - C).astype(np.float16)
    ceff = cw1 - bias16.astype(np.float32)
    G0 = ceff.max()
    E = np.exp(ceff - G0).astype(np.float32)

    in_maps = []
    for k in range(N_CORES):
        sl = slice(k * R, (k + 1) * R)
        cshard = np.ascontiguousarray(context[sl])
        caugT = np.empty((102, R), np.float16)
        caugT[0:D] = cshard.T
        caugT[D] = 1.0
        caugT[D + 1] = bias16[sl]
        efac = np.ascontiguousarray(
            E[sl].reshape(NCH, P).T.astype(np.float32))
        in_maps.append({
            "ctx": cshard,
            "caugT": np.ascontiguousarray(caugT),
            "qaugT": qaugT,
            "qnr": qnr,
            "efac": efac,
            "ident": ident,
        })
    return in_maps


def kernel(**inputs):
    from concourse.bass_utils import run_bass_kernel_spmd

    in_maps = _prep_inputs(inputs)
    res = run_bass_kernel_spmd(_get_nc(), in_maps,
                               core_ids=list(range(N_CORES)))
    return np.concatenate([res.results[k]["g"] for k in range(N_CORES)],
                          axis=0)


def _install_ntff_hook():
    """This image's antenv lacks axon_hooks, so trn_boot's NTFF profile
    hook install degrades silently. Recreate the glue module and install
    the ctypes hook so run_bass_kernel_spmd(trace=True) works."""
    import sys
    import types

    if "antenv.axon_hooks" not in sys.modules:
        import antenv

        mod = types.ModuleType("antenv.axon_hooks")
        mod._hook = None

        def set_axon_ntff_profile_hook(h):
            mod._hook = h

        def get_axon_ntff_profile_hook():
            return mod._hook

        mod.set_axon_ntff_profile_hook = set_axon_ntff_profile_hook
        mod.get_axon_ntff_profile_hook = get_axon_ntff_profile_hook
        sys.modules["antenv.axon_hooks"] = mod
        antenv.axon_hooks = mod
    m = sys.modules["antenv.axon_hooks"]
    if m._hook is None:
        from trn_agent_boot.trn_boot import _ntff_profile_via_ctypes

        m.set_axon_ntff_profile_hook(
            _ntff_profile_via_ctypes("/opt/axon/libaxon_pjrt.so"))


def kernel_traced(**inputs):
    """Like kernel() but also returns HW exec time in ns (NTFF profile)."""
    import os
    import shutil

    from concourse.bass_utils import run_bass_kernel_spmd

    out = kernel(**inputs)  # warm compile via cached nc
    _install_ntff_hook()
    tracedir = "/root/problem/work/trace"
    shutil.rmtree(tracedir, ignore_errors=True)
    os.makedirs(tracedir, exist_ok=True)
    in_maps = _prep_inputs(inputs)
    res = run_bass_kernel_spmd(_get_nc(), in_maps,
                               core_ids=list(range(N_CORES)), trace=True,
                               tmpdir=tracedir)
    out = np.concatenate([res.results[k]["g"] for k in range(N_CORES)],
                         axis=0)
    return out, res.exec_time_ns
